# revision 10
# baseline (speedup 1.0000x reference)
"""Trainium2 8-core kernel for nn_AttnAgg (sparse attention aggregation).

Math (see reference):
  Q = main @ Wq.T + bq                     [2048, 512]
  K = other @ Wk.T + bk                    [2048, 512]
  attn = softmax(where(mask, -BIG, Q K.T / sqrt(512)), axis=-1)   [2048, 2048]
  out[b, m, k] = sum_o attn[m, o] * fix[b, o] * other[o, k]       [32, 2048, 512]

Sharding: rows of `main` (the m axis) are split 256-per-core across 8 cores —
attention and the big einsum shard perfectly with zero collectives; only the
K projection (~1 GFLOP) is replicated.

The dominant cost is the batched aggregation einsum (B*M*O*D = 137 GFLOP of
the ~144 GFLOP total).  It runs in fp8e4 with perf_mode=DoubleRow (2 fp8
MACs per PE cell per cycle; the DR matmuls issue at the 512-cycle streaming
floor).  Straight fp8 fails the 2e-2 tolerance (measured 2.9e-2), so the
batch coupling `fix` is mean/delta decomposed on the host:
fix[b,o] = mu[o] + delta[b,o].  The batch-independent mu-term
(p @ bf16(mu*other)) is ONE extra bf16 matmul pass (1/32 of the einsum
work); only the delta-term runs in fp8, and |p*delta| is ~half |p*fix|,
which halves the fp8 noise (measured 1.29e-2).  The softmax denominator
comes from the same bf16 p (matmul with ones), so normalization is
consistent.  Projections run on bf16 inputs; the logits matmul stays
float32r (fp8 anywhere in the projections/logits measured >2e-2 in
numpy simulation — don't retry).

Per-batch steady state is balanced three ways at ~3.8us (measured op costs:
DVE merged TT = (elems+151)/0.96ns, ACT fp8-out chunk ~635ns, copies ~0.7us):
  PE:  1 identity mean-add MM (mt0 only) + 16 DoubleRow MMs  ~3.7us
  DVE: merged 3D tensor_tensor wf[0:11] = pt * delta (stride-0 broadcast
       delta column, runs at 1x - fp32/broadcast blocks the 2x uop), plus
       the mt1 output STT copy (ps*recip + meanR in ONE op)       ~3.8us
  ACT: wf[11:16] (5 per-chunk activations; fp8-out costs 2cyc/elem on ACT),
       plus the mt0 output copy                                    ~3.8us
mt1's mean term is added during the copy-out (scalar_tensor_tensor with
per-partition recip scalar and a precomputed f32 meanR = psm*recip), which
drops its identity matmul from the PE.  mt0 keeps the identity-MM trick
(ACT's activation cannot add a full-tensor bias).  GPSIMD measured 15x
slower than DVE for elementwise fp8 — not usable for wf.

Emission is software-pipelined LOOKAHEAD batches ahead; additionally wf for
batches 0-1 is emitted BEFORE the recip/mean copies so DVE/ACT produce them
during the ~7us rowsum/mean matmul window (engine queues are strict FIFO —
this ordering is load-bearing).

DMA: descriptor issue costs ~650ns per dma_start ON THE ISSUING ENGINE'S
QUEUE (measured), so input loads are consolidated into few, large DMAs and
spread across the sync/gpsimd/vector/scalar queues to issue in parallel
right after the ~6us engine-preamble barrier.  PE warmup matmuls (HAM
clock-gate) gate on a DVE memset tile, not on input DMA, so the ramp starts
at ~5.6us.  Output stores: GB-batch groups, mt0 issued from sync / mt1 from
gpsimd (parallel); the last group is split pair+single+single so the final
transfer is small and the serial ~650ns issues overlap earlier compute.

Inputs are fed pre-transposed AND partition-packed: every DRAM tensor is
laid out [128, *] so that each SBUF partition's data is one long contiguous
DRAM run.  A tile row-block T of a logical [T*128, W] matrix lives at
packed[:, T*W:(T+1)*W]; for DoubleRow the pair dim indexes adjacent 128-row
blocks of the contraction (o) axis.
"""

import math
import os
import sys

import numpy as np

if "/opt/trn_rl_repo" not in sys.path:
    sys.path.insert(0, "/opt/trn_rl_repo")

import ml_dtypes

import concourse.bass as bass
import concourse.tile as tile
from concourse import bacc, mybir
from concourse.bass_utils import run_bass_kernel_spmd

F32 = mybir.dt.float32
F32R = mybir.dt.float32r
BF16 = mybir.dt.bfloat16
F8 = mybir.dt.float8e4
U8 = mybir.dt.uint8
AF = mybir.ActivationFunctionType
DR = mybir.MatmulPerfMode.DoubleRow
MUL = mybir.AluOpType.mult
ADD = mybir.AluOpType.add

N_CORES = 8
M, O, D = 2048, 2048, 512       # main rows, other rows, qdim=kdim=mid
B = 32                          # batch
MC = M // N_CORES               # 256 main rows per core
P = 128
GB = 4                          # batches per output store DMA
N_WARM = 42                     # dummy matmuls to warm the PE clock gate
N_WF_DVE = 12                   # wf chunks (of 16) on DVE (one merged op)
LOOKAHEAD = 5                   # extra wf batches beyond the 2 pre-produced
TAIL_PB = 8                     # per-batch output stores for the last 8

_CACHE = {}
LAST_RESULTS = None             # test harness reads exec_time_ns from here


def _build():
    nc = bacc.Bacc("TRN2", target_bir_lowering=False, debug=False,
                   num_devices=N_CORES)

    NDT = D // P                # 4 tiles along the 512 dims
    NOT = O // P                # 16 tiles along o
    NMT = MC // P               # 2 tiles along m

    d_mainT = nc.dram_tensor("mainT", [P, NDT * MC], BF16,
                             kind="ExternalInput").ap()
    d_wqT = nc.dram_tensor("wqT", [P, NDT * D], BF16,
                           kind="ExternalInput").ap()
    d_wkT = nc.dram_tensor("wkT", [P, NDT * D], BF16,
                           kind="ExternalInput").ap()
    d_bias = nc.dram_tensor("bias", [P, 2 * NDT], F32,
                            kind="ExternalInput").ap()   # bq || bk
    d_otherT = nc.dram_tensor("otherT", [P, NDT * O], BF16,
                              kind="ExternalInput").ap()   # fc-major
    d_other8 = nc.dram_tensor("other8", [P, NOT * D], F8,
                              kind="ExternalInput").ap()   # ot-major, fp8
    d_otherM = nc.dram_tensor("otherM", [P, NOT * D], BF16,
                              kind="ExternalInput").ap()   # mu*other, bf16
    d_deltaT = nc.dram_tensor("deltaT", [P, NOT * B], F32,
                              kind="ExternalInput").ap()   # fix - mu
    d_maskT = nc.dram_tensor("maskT", [P, NOT * MC], U8,
                             kind="ExternalInput").ap()
    d_ident = nc.dram_tensor("ident", [P, P], BF16,
                             kind="ExternalInput").ap()
    d_out = nc.dram_tensor("out", [MC, B, D], BF16,
                           kind="ExternalOutput").ap()

    with tile.TileContext(nc) as tc:
        with tc.tile_pool(name="persist", bufs=1) as pp, \
             tc.tile_pool(name="wpool", bufs=16) as wpool, \
             tc.tile_pool(name="outp", bufs=4) as outp:

            # ---- loads: few big DMAs, issued in parallel across four
            # engine queues (each dma_start costs ~650ns of issue time on
            # its queue).  Ordered by need-time within each queue. --------
            with tc.tile_pool(name="proj", bufs=1) as proj, \
                 tc.tile_pool(name="psqk", bufs=2, space="PSUM") as psqk:
                # ---- PE warmup (emitted FIRST so the DVE memset is the
                # head of the vector queue): dummy matmuls gated on a
                # memset tile, NOT on input DMA — the HAM ramp starts
                # right after the engine preamble (~5.6us), so the
                # clock-gate is at 8/8 before real work begins.
                warmP = proj.tile([P, P], BF16, name="warmP", tag="warmP")
                nc.vector.memset(warmP[:], 0.125)
                warm_ps = psqk.tile([P, P], F32, name="warm_ps", tag="warm",
                                    bufs=1)
                for _ in range(N_WARM):
                    nc.tensor.matmul(warm_ps[:], warmP[:], warmP[:],
                                     start=True, stop=True)

                # ALL input loads on the single sync queue, strictly in
                # need order: concurrent multi-queue input DMAs measured
                # SLOWER — they flood HBM and the critical-path tensors
                # (wk/ot for the K projection) drop to a 1/3 bandwidth
                # share, starving the PE and re-throttling the HAM clock
                # gate.  Serial issue on one queue = transfers get full
                # bandwidth in exactly this order.  fc/ct-granular splits
                # give the K projection chunk-level semaphores so it can
                # start/progress as data lands.
                biasP = proj.tile([P, 2 * NDT], F32, name="biasP",
                                  tag="biasP")
                nc.sync.dma_start(biasP[:], d_bias[:])
                wqP = proj.tile([P, NDT * D], BF16, name="wqP", tag="wqP")
                nc.sync.dma_start(wqP[:], d_wqT[:])
                mtP = proj.tile([P, NDT * MC], BF16, name="mtP", tag="mtP")
                nc.sync.dma_start(mtP[:], d_mainT[:])
                wkP = proj.tile([P, NDT * D], BF16, name="wkP", tag="wkP")
                nc.sync.dma_start(wkP[:], d_wkT[:])
                otP = proj.tile([P, NDT * O], BF16, name="otP", tag="otP")
                for ct in range(NDT):
                    nc.sync.dma_start(otP[:, ct * D:(ct + 1) * D],
                                      d_otherT[:, ct * D:(ct + 1) * D])
                for fc in range(1, NDT):
                    nc.sync.dma_start(otP[:, fc * O:(fc + 1) * O],
                                      d_otherT[:, fc * O:(fc + 1) * O])
                maskP = pp.tile([P, NOT, MC], U8, name="maskP", tag="maskP")
                nc.sync.dma_start(maskP[:], d_maskT[:])
                otherMP = pp.tile([P, NOT, D], BF16, name="otherMP",
                                  tag="otherMP")
                nc.sync.dma_start(otherMP[:], d_otherM[:])
                otherP = pp.tile([P, NOT, D], F8, name="otherP",
                                 tag="otherP")
                nc.sync.dma_start(otherP[:], d_other8[:])
                deltaP = pp.tile([P, NOT, B], F32, name="deltaP",
                                 tag="deltaP")
                nc.sync.dma_start(deltaP[:], d_deltaT[:])
                identP = pp.tile([P, P], BF16, name="identP", tag="identP")
                nc.sync.dma_start(identP[:], d_ident[:])

                qt_sb = [pp.tile([P, MC], F32, name=f"qt{i}", tag=f"qt{i}")
                         for i in range(NDT)]
                kt_sb = [pp.tile([P, O], F32, name=f"kt{i}", tag=f"kt{i}")
                         for i in range(NDT)]
                pt_all = pp.tile([P, NOT, MC], BF16, name="pt", tag="pt")
                ones_sb = pp.tile([P, 1], BF16, name="ones", tag="ones")
                nc.vector.memset(ones_sb[:], 1.0)
                recip_sb = [pp.tile([P, 1], F32, name=f"recip{i}",
                                    tag=f"recip{i}") for i in range(NMT)]
                mean_sb = [pp.tile([P, D], BF16, name=f"mean{i}",
                                   tag=f"mean{i}") for i in range(NMT)]

                # ---- KT fc0 first (earliest DMAs), then QT, then rest
                def emit_qt():
                    for pt in range(NDT):
                        ps = psqk.tile([P, MC], F32, name="psq", tag="psq")
                        for ct in range(NDT):
                            nc.tensor.matmul(
                                ps[:],
                                wqP[:, ct * D + pt * P:ct * D + (pt + 1) * P],
                                mtP[:, ct * MC:(ct + 1) * MC],
                                start=(ct == 0), stop=(ct == NDT - 1))
                        nc.scalar.activation(qt_sb[pt][:].bitcast(F32R),
                                             ps[:], AF.Identity,
                                             bias=biasP[:, pt:pt + 1])

                def emit_attn(op):
                    # logits for ot pair (2op, 2op+1), mask, exp
                    ps = psqk.tile([P, 2, MC], F32, name="psa", tag="psa")
                    for j in range(2):
                        ot = 2 * op + j
                        for ct in range(NDT):
                            nc.tensor.matmul(
                                ps[:, j, :],
                                kt_sb[ct][:, ot * P:(ot + 1) * P]
                                .bitcast(F32R),
                                qt_sb[ct][:].bitcast(F32R),
                                start=(ct == 0), stop=(ct == NDT - 1))
                    # psa += mask * -1e9  (u8 -> f32 convert, scale, add in
                    # one DVE pass); exp underflows masked lanes to exact 0
                    nc.vector.scalar_tensor_tensor(
                        ps[:], maskP[:, 2 * op:2 * op + 2, :], -1.0e9, ps[:],
                        op0=MUL, op1=ADD)
                    nc.scalar.activation(pt_all[:, 2 * op:2 * op + 2, :],
                                         ps[:], AF.Exp)

                # QT first (its inputs are the first big DMAs), then each
                # KT fc chunk immediately followed by the two attention ot
                # pairs it unlocks — the PE is never more than one DMA
                # chunk ahead of the data stream, and the mask/exp tail
                # spreads across the whole projection phase instead of
                # bunching at its end.
                emit_qt()
                for fc in range(NDT):
                    for pt in range(NDT):
                        ps = psqk.tile([P, D], F32, name="psk", tag="psk")
                        for ct in range(NDT):
                            nc.tensor.matmul(
                                ps[:],
                                wkP[:, ct * D + pt * P:ct * D + (pt + 1) * P],
                                otP[:, fc * O + ct * D:fc * O + (ct + 1) * D],
                                start=(ct == 0), stop=(ct == NDT - 1))
                        nc.scalar.activation(
                            kt_sb[pt][:, fc * D:(fc + 1) * D].bitcast(F32R),
                            ps[:], AF.Identity,
                            bias=biasP[:, NDT + pt:NDT + pt + 1])
                    emit_attn(2 * fc)
                    emit_attn(2 * fc + 1)

            # ---- attnT, exp, rowsum -----------------------------------
            # ps4 (attn: 2 + rowsum: 2 banks) and pso (out: 4 banks) coexist
            # so the first batch's matmuls need not wait for the softmax
            # tail to release PSUM — otherwise the PE goes idle long enough
            # mid-kernel for the HAM clock-gate to re-throttle it.
            with tc.tile_pool(name="ps4", bufs=2, space="PSUM") as ps4:
                # ---- rowsum + mean term, interleaved per-ot so the
                # rowsum's LDWEIGHTS hide under the mean matmuls' streaming
                psr = [ps4.tile([P, 1], F32, name=f"psr{mt}", tag=f"psr{mt}",
                                bufs=1) for mt in range(NMT)]
                psm = [ps4.tile([P, D], F32, name=f"psm{mt}", tag=f"psm{mt}",
                                bufs=1) for mt in range(NMT)]
                for ot in range(NOT):
                    for mt in range(NMT):
                        nc.tensor.matmul(
                            psr[mt][:],
                            pt_all[:, ot, mt * P:(mt + 1) * P],
                            ones_sb[:],
                            start=(ot == 0), stop=(ot == NOT - 1))
                        nc.tensor.matmul(
                            psm[mt][:],
                            pt_all[:, ot, mt * P:(mt + 1) * P],
                            otherMP[:, ot, :],
                            start=(ot == 0), stop=(ot == NOT - 1))

                # ---- wf production + softmax epilogue -----------------
                # wf ops for batch b enter the (in-order) DVE/ACT queues
                # BEFORE the psum->SBUF copies of batch b-1, so a copy
                # stalled on the PE never blocks wf production.
                osb = {}
                wfs = {}

                NA = NOT - N_WF_DVE     # ACT chunks: 0..NA-1

                def emit_wf(b):
                    # two separate tiles so the DVE and ACT write streams
                    # have no common tile and never serialize on it.  ACT
                    # takes the FIRST chunks (their exps finish earliest,
                    # so ACT's strict-FIFO queue never blocks on a late
                    # exp); DVE's merged op takes the rest.
                    wfb = wpool.tile([P, NA, MC], F8, name="wfb", tag="wfb")
                    wfa = wpool.tile([P, N_WF_DVE, MC], F8, name="wfa",
                                     tag="wfa")
                    wfs[b] = (wfa, wfb)
                    for ot in range(NA):
                        nc.scalar.activation(
                            wfb[:, ot, :], pt_all[:, ot, :],
                            AF.Copy, scale=deltaP[:, ot, b:b + 1])
                    # DVE: one merged 3D op with stride-0 broadcast delta
                    nc.vector.tensor_tensor(
                        wfa[:], pt_all[:, NA:NOT, :],
                        deltaP[:, NA:NOT, b:b + 1]
                        .to_broadcast([P, N_WF_DVE, MC]), MUL)

                # wf for b=0,1 BEFORE the recip/mean ops: DVE/ACT chew them
                # during the rowsum/mean matmul window, and the recip/mean
                # ops (which agg b0 gates on) become ready right as psr/psm
                # complete.
                emit_wf(0)
                emit_wf(1)

                for mt in range(NMT):
                    nc.vector.reciprocal(recip_sb[mt][:], psr[mt][:])
                    nc.scalar.activation(mean_sb[mt][:], psm[mt][:], AF.Copy)

            # ---- weighted aggregation (fp8 DoubleRow) -----------------
            with tc.tile_pool(name="pso", bufs=8, space="PSUM") as psop:

                def emit_agg(b):
                    wfa, wfb = wfs.pop(b)
                    for mt in range(NMT):
                        if b % GB == 0:
                            osb[mt] = outp.tile([P, GB * D], BF16,
                                                name="osb", tag=f"osb{mt}")
                        ps = psop.tile([P, D], F32, name="pso", tag="pso")
                        # open the group with psum = mean (identity MM),
                        # then accumulate the fp8 delta-term on top
                        nc.tensor.matmul(ps[:], identP[:], mean_sb[mt][:],
                                         start=True, stop=False,
                                         skip_group_check=True)
                        for op in range(NOT // 2):
                            hi = 2 * op >= NA
                            w = wfa if hi else wfb
                            o0 = 2 * op - (NA if hi else 0)
                            nc.tensor.matmul(
                                ps[:],
                                w[:, o0:o0 + 2, mt * P:(mt + 1) * P],
                                otherP[:, 2 * op:2 * op + 2, :],
                                start=False, stop=(op == NOT // 2 - 1),
                                perf_mode=DR, skip_group_check=True)
                        j = b % GB
                        # BOTH copies on ACT: DVE's merged wf op is its
                        # whole per-batch budget (3.35us); ACT has slack.
                        nc.scalar.activation(
                            osb[mt][:, j * D:(j + 1) * D], ps[:],
                            AF.Copy, scale=recip_sb[mt][:])
                        # stores: mt0 via sync, mt1 via gpsimd (parallel
                        # issue queues).  Groups of GB until the tail; the
                        # last group goes pair+single+single so the final
                        # post-compute DMA is small and issues overlap.
                        eng = nc.sync if mt == 0 else nc.gpsimd
                        if b < B - GB:
                            if j == GB - 1:
                                eng.dma_start(
                                    d_out[mt * P:(mt + 1) * P,
                                          b - GB + 1:b + 1, :], osb[mt][:])
                        elif b == B - 3:
                            eng.dma_start(
                                d_out[mt * P:(mt + 1) * P, B - GB:B - 2, :],
                                osb[mt][:, 0:2 * D])
                        elif b >= B - 2:
                            eng.dma_start(
                                d_out[mt * P:(mt + 1) * P, b:b + 1, :],
                                osb[mt][:, j * D:(j + 1) * D])

                for b in range(2, 2 + LOOKAHEAD):
                    emit_wf(b)
                for b in range(B):
                    if b + LOOKAHEAD + 2 < B:
                        emit_wf(b + LOOKAHEAD + 2)
                    emit_agg(b)

    nc.compile()
    return nc


def _pack(a, ntiles, width):
    """[ntiles*128, width] -> [128, ntiles*width] partition-packed layout."""
    return np.ascontiguousarray(
        a.reshape(ntiles, P, width).transpose(1, 0, 2).reshape(P, -1))


def kernel(main_feat, other_feat, fix_feat, mask, Wq, bq, Wk, bk):
    global LAST_RESULTS
    main_feat = np.asarray(main_feat, dtype=np.float32)
    other_feat = np.asarray(other_feat, dtype=np.float32)
    fix_feat = np.asarray(fix_feat, dtype=np.float32)
    mask = np.asarray(mask)
    Wq = np.asarray(Wq, dtype=np.float32)
    bq = np.asarray(bq, dtype=np.float32)
    Wk = np.asarray(Wk, dtype=np.float32)
    bk = np.asarray(bk, dtype=np.float32)

    if "nc" not in _CACHE:
        _CACHE["nc"] = _build()
    nc = _CACHE["nc"]

    NDT, NOT = D // P, O // P
    inv = np.float32(1.0 / math.sqrt(D))
    wqT = _pack(Wq.T * inv, NDT, D).astype(ml_dtypes.bfloat16)
    bq_p = _pack((bq * inv).reshape(D, 1), NDT, 1)
    wkT = _pack(np.ascontiguousarray(Wk.T), NDT, D).astype(
        ml_dtypes.bfloat16)
    bk_p = _pack(bk.reshape(D, 1), NDT, 1)
    bias = np.concatenate([bq_p, bk_p], axis=1)       # [P, 2*NDT] f32
    # otherT fc-major: [p, fc*O + ct*D + oo] = other.T[ct*128+p, fc*D+oo]
    otherT = np.ascontiguousarray(
        other_feat.T.reshape(NDT, P, NDT, D).transpose(1, 2, 0, 3)
        .reshape(P, NDT * O)).astype(ml_dtypes.bfloat16)
    other8 = _pack(other_feat, NOT, D).astype(ml_dtypes.float8_e4m3)
    mu = fix_feat.mean(axis=0)                        # [O]
    otherM = _pack(mu[:, None] * other_feat, NOT, D).astype(
        ml_dtypes.bfloat16)
    deltaT = _pack(np.ascontiguousarray((fix_feat - mu).T), NOT, B)
    ident = np.eye(P, dtype=np.float32).astype(ml_dtypes.bfloat16)
    mainT = main_feat.T                               # [D, M] view
    mask_u8 = mask.astype(np.uint8)                   # [M, O]

    in_maps = []
    for c in range(N_CORES):
        sl = slice(c * MC, (c + 1) * MC)
        in_maps.append({
            "mainT": _pack(np.ascontiguousarray(mainT[:, sl]),
                           NDT, MC).astype(ml_dtypes.bfloat16),
            "wqT": wqT, "wkT": wkT, "bias": bias,
            "otherT": otherT, "other8": other8, "otherM": otherM,
            "deltaT": deltaT, "ident": ident,
            "maskT": _pack(np.ascontiguousarray(mask_u8[sl, :].T), NOT, MC),
        })

    try:
        res = run_bass_kernel_spmd(nc, in_maps, core_ids=list(range(N_CORES)))
    except Exception:
        # The BASS_TRACE=1 profiling path needs antenv.axon_hooks + artifact
        # upload, which not every image carries — rerun without tracing.
        if os.environ.get("BASS_NEVER_TRACE") == "1":
            raise
        os.environ["BASS_NEVER_TRACE"] = "1"
        res = run_bass_kernel_spmd(nc, in_maps, core_ids=list(range(N_CORES)))
    LAST_RESULTS = res
    # device layout is [MC, B, D] per core -> [B, MC, D], concat on m
    return np.concatenate(
        [res.results[c]["out"].transpose(1, 0, 2) for c in range(N_CORES)],
        axis=1).astype(np.float32)


# revision 11
# speedup vs baseline: 1.1764x; 1.1764x over previous
"""Trainium2 8-core kernel for nn_AttnAgg (sparse attention aggregation).

Math (see reference):
  Q = main @ Wq.T + bq                     [2048, 512]
  K = other @ Wk.T + bk                    [2048, 512]
  attn = softmax(where(mask, -BIG, Q K.T / sqrt(512)), axis=-1)   [2048, 2048]
  out[b, m, k] = sum_o attn[m, o] * fix[b, o] * other[o, k]       [32, 2048, 512]

Sharding: rows of `main` (the m axis) are split 256-per-core across 8 cores —
attention and the big einsum shard perfectly with zero collectives; only the
K projection (~1 GFLOP) is replicated.

The dominant cost is the batched aggregation einsum (B*M*O*D = 137 GFLOP of
the ~144 GFLOP total).  It runs in fp8e4 with perf_mode=DoubleRow (2 fp8
MACs per PE cell per cycle; the DR matmuls issue at the 512-cycle streaming
floor).  Straight fp8 fails the 2e-2 tolerance (measured 2.9e-2), so the
batch coupling `fix` is mean/delta decomposed on the host:
fix[b,o] = mu[o] + delta[b,o].  The batch-independent mu-term
(p @ bf16(mu*other)) is ONE extra bf16 matmul pass (1/32 of the einsum
work); only the delta-term runs in fp8, and |p*delta| is ~half |p*fix|,
which halves the fp8 noise (measured 1.29e-2).  The softmax denominator
comes from the same bf16 p (matmul with ones), so normalization is
consistent.  Projections run on bf16 inputs; the logits matmul stays
float32r (fp8 anywhere in the projections/logits measured >2e-2 in
numpy simulation — don't retry).

Per-batch steady state is balanced three ways at ~3.8us (measured op costs:
DVE merged TT = (elems+151)/0.96ns, ACT fp8-out chunk ~635ns, copies ~0.7us):
  PE:  1 identity mean-add MM (mt0 only) + 16 DoubleRow MMs  ~3.7us
  DVE: merged 3D tensor_tensor wf[0:11] = pt * delta (stride-0 broadcast
       delta column, runs at 1x - fp32/broadcast blocks the 2x uop), plus
       the mt1 output STT copy (ps*recip + meanR in ONE op)       ~3.8us
  ACT: wf[11:16] (5 per-chunk activations; fp8-out costs 2cyc/elem on ACT),
       plus the mt0 output copy                                    ~3.8us
mt1's mean term is added during the copy-out (scalar_tensor_tensor with
per-partition recip scalar and a precomputed f32 meanR = psm*recip), which
drops its identity matmul from the PE.  mt0 keeps the identity-MM trick
(ACT's activation cannot add a full-tensor bias).  GPSIMD measured 15x
slower than DVE for elementwise fp8 — not usable for wf.

Emission is software-pipelined LOOKAHEAD batches ahead; additionally wf for
batches 0-1 is emitted BEFORE the recip/mean copies so DVE/ACT produce them
during the ~7us rowsum/mean matmul window (engine queues are strict FIFO —
this ordering is load-bearing).

DMA: descriptor issue costs ~650ns per dma_start ON THE ISSUING ENGINE'S
QUEUE (measured), so input loads are consolidated into few, large DMAs and
spread across the sync/gpsimd/vector/scalar queues to issue in parallel
right after the ~6us engine-preamble barrier.  PE warmup matmuls (HAM
clock-gate) gate on a DVE memset tile, not on input DMA, so the ramp starts
at ~5.6us.  Output stores: GB-batch groups, mt0 issued from sync / mt1 from
gpsimd (parallel); the last group is split pair+single+single so the final
transfer is small and the serial ~650ns issues overlap earlier compute.

Inputs are fed pre-transposed AND partition-packed: every DRAM tensor is
laid out [128, *] so that each SBUF partition's data is one long contiguous
DRAM run.  A tile row-block T of a logical [T*128, W] matrix lives at
packed[:, T*W:(T+1)*W]; for DoubleRow the pair dim indexes adjacent 128-row
blocks of the contraction (o) axis.
"""

import math
import os
import sys

import numpy as np

if "/opt/trn_rl_repo" not in sys.path:
    sys.path.insert(0, "/opt/trn_rl_repo")

import ml_dtypes

import concourse.bass as bass
import concourse.tile as tile
from concourse import bacc, mybir
from concourse.bass_utils import run_bass_kernel_spmd

F32 = mybir.dt.float32
F32R = mybir.dt.float32r
BF16 = mybir.dt.bfloat16
F8 = mybir.dt.float8e4
U8 = mybir.dt.uint8
AF = mybir.ActivationFunctionType
DR = mybir.MatmulPerfMode.DoubleRow
MUL = mybir.AluOpType.mult
ADD = mybir.AluOpType.add

N_CORES = 8
M, O, D = 2048, 2048, 512       # main rows, other rows, qdim=kdim=mid
B = 32                          # batch
MC = M // N_CORES               # 256 main rows per core
P = 128
GB = 4                          # batches per output store DMA
N_WARM = 42                     # dummy matmuls to warm the PE clock gate
N_WF_DVE = 12                   # wf chunks (of 16) on DVE (one merged op)
LOOKAHEAD = 5                   # extra wf batches beyond the 2 pre-produced
TAIL_PB = 8                     # per-batch output stores for the last 8

_CACHE = {}
LAST_RESULTS = None             # test harness reads exec_time_ns from here


def _build():
    nc = bacc.Bacc("TRN2", target_bir_lowering=False, debug=False,
                   num_devices=N_CORES)

    NDT = D // P                # 4 tiles along the 512 dims
    NOT = O // P                # 16 tiles along o
    NMT = MC // P               # 2 tiles along m

    d_mainT = nc.dram_tensor("mainT", [P, NDT * MC], BF16,
                             kind="ExternalInput").ap()
    d_wqT = nc.dram_tensor("wqT", [P, NDT * D], BF16,
                           kind="ExternalInput").ap()
    d_wkT = nc.dram_tensor("wkT", [P, NDT * D], BF16,
                           kind="ExternalInput").ap()
    d_bias = nc.dram_tensor("bias", [P, 2 * NDT], F32,
                            kind="ExternalInput").ap()   # bq || bk
    d_otherT = nc.dram_tensor("otherT", [P, NDT * O], BF16,
                              kind="ExternalInput").ap()   # fc-major
    d_other8 = nc.dram_tensor("other8", [P, NOT * D], F8,
                              kind="ExternalInput").ap()   # ot-major, fp8
    d_otherM = nc.dram_tensor("otherM", [P, NOT * D], BF16,
                              kind="ExternalInput").ap()   # mu*other, bf16
    d_deltaT = nc.dram_tensor("deltaT", [P, NOT * B], F32,
                              kind="ExternalInput").ap()   # fix - mu
    d_maskT = nc.dram_tensor("maskT", [P, NOT * MC], U8,
                             kind="ExternalInput").ap()
    d_ident = nc.dram_tensor("ident", [P, P], BF16,
                             kind="ExternalInput").ap()
    d_out = nc.dram_tensor("out", [MC, B, D], BF16,
                           kind="ExternalOutput").ap()

    with tile.TileContext(nc) as tc:
        with tc.tile_pool(name="persist", bufs=1) as pp, \
             tc.tile_pool(name="wpool", bufs=16) as wpool, \
             tc.tile_pool(name="outp", bufs=4) as outp:

            # ---- loads: few big DMAs, issued in parallel across four
            # engine queues (each dma_start costs ~650ns of issue time on
            # its queue).  Ordered by need-time within each queue. --------
            with tc.tile_pool(name="proj", bufs=1) as proj, \
                 tc.tile_pool(name="psqk", bufs=2, space="PSUM") as psqk:
                # ---- PE warmup (emitted FIRST so the DVE memset is the
                # head of the vector queue): dummy matmuls gated on a
                # memset tile, NOT on input DMA — the HAM ramp starts
                # right after the engine preamble (~5.6us), so the
                # clock-gate is at 8/8 before real work begins.
                warmP = proj.tile([P, P], BF16, name="warmP", tag="warmP")
                nc.vector.memset(warmP[:], 0.125)
                warm_ps = psqk.tile([P, P], F32, name="warm_ps", tag="warm",
                                    bufs=1)
                for _ in range(N_WARM):
                    nc.tensor.matmul(warm_ps[:], warmP[:], warmP[:],
                                     start=True, stop=True)

                # ALL input loads on the single sync queue, strictly in
                # need order: concurrent multi-queue input DMAs measured
                # SLOWER — they flood HBM and the critical-path tensors
                # (wk/ot for the K projection) drop to a 1/3 bandwidth
                # share, starving the PE and re-throttling the HAM clock
                # gate.  Serial issue on one queue = transfers get full
                # bandwidth in exactly this order.  fc/ct-granular splits
                # give the K projection chunk-level semaphores so it can
                # start/progress as data lands.
                biasP = proj.tile([P, 2 * NDT], F32, name="biasP",
                                  tag="biasP")
                nc.sync.dma_start(biasP[:], d_bias[:])
                wqP = proj.tile([P, NDT * D], BF16, name="wqP", tag="wqP")
                nc.sync.dma_start(wqP[:], d_wqT[:])
                mtP = proj.tile([P, NDT * MC], BF16, name="mtP", tag="mtP")
                nc.sync.dma_start(mtP[:], d_mainT[:])
                wkP = proj.tile([P, NDT * D], BF16, name="wkP", tag="wkP")
                nc.sync.dma_start(wkP[:], d_wkT[:])
                otP = proj.tile([P, NDT * O], BF16, name="otP", tag="otP")
                for ct in range(NDT):
                    nc.sync.dma_start(otP[:, ct * D:(ct + 1) * D],
                                      d_otherT[:, ct * D:(ct + 1) * D])
                for fc in range(1, NDT):
                    nc.sync.dma_start(otP[:, fc * O:(fc + 1) * O],
                                      d_otherT[:, fc * O:(fc + 1) * O])
                maskP = pp.tile([P, NOT, MC], U8, name="maskP", tag="maskP")
                nc.sync.dma_start(maskP[:], d_maskT[:])
                otherMP = pp.tile([P, NOT, D], BF16, name="otherMP",
                                  tag="otherMP")
                nc.sync.dma_start(otherMP[:], d_otherM[:])
                otherP = pp.tile([P, NOT, D], F8, name="otherP",
                                 tag="otherP")
                nc.sync.dma_start(otherP[:], d_other8[:])
                deltaP = pp.tile([P, NOT, B], F32, name="deltaP",
                                 tag="deltaP")
                nc.sync.dma_start(deltaP[:], d_deltaT[:])
                identP = pp.tile([P, P], BF16, name="identP", tag="identP")
                nc.sync.dma_start(identP[:], d_ident[:])

                qt_sb = [pp.tile([P, MC], F32, name=f"qt{i}", tag=f"qt{i}")
                         for i in range(NDT)]
                kt_sb = [pp.tile([P, O], F32, name=f"kt{i}", tag=f"kt{i}")
                         for i in range(NDT)]
                pt_all = pp.tile([P, NOT, MC], BF16, name="pt", tag="pt")
                ones_sb = pp.tile([P, 1], BF16, name="ones", tag="ones")
                nc.vector.memset(ones_sb[:], 1.0)
                recip_sb = [pp.tile([P, 1], F32, name=f"recip{i}",
                                    tag=f"recip{i}") for i in range(NMT)]
                mean_sb = [pp.tile([P, D], BF16, name=f"mean{i}",
                                   tag=f"mean{i}") for i in range(NMT)]

                # ---- KT fc0 first (earliest DMAs), then QT, then rest
                def emit_qt():
                    for pt in range(NDT):
                        ps = psqk.tile([P, MC], F32, name="psq", tag="psq")
                        for ct in range(NDT):
                            nc.tensor.matmul(
                                ps[:],
                                wqP[:, ct * D + pt * P:ct * D + (pt + 1) * P],
                                mtP[:, ct * MC:(ct + 1) * MC],
                                start=(ct == 0), stop=(ct == NDT - 1))
                        nc.scalar.activation(qt_sb[pt][:].bitcast(F32R),
                                             ps[:], AF.Identity,
                                             bias=biasP[:, pt:pt + 1])

                def emit_attn(op):
                    # logits for ot pair (2op, 2op+1), mask, exp
                    ps = psqk.tile([P, 2, MC], F32, name="psa", tag="psa")
                    for j in range(2):
                        ot = 2 * op + j
                        for ct in range(NDT):
                            nc.tensor.matmul(
                                ps[:, j, :],
                                kt_sb[ct][:, ot * P:(ot + 1) * P]
                                .bitcast(F32R),
                                qt_sb[ct][:].bitcast(F32R),
                                start=(ct == 0), stop=(ct == NDT - 1))
                    # psa += mask * -1e9  (u8 -> f32 convert, scale, add in
                    # one DVE pass); exp underflows masked lanes to exact 0
                    nc.vector.scalar_tensor_tensor(
                        ps[:], maskP[:, 2 * op:2 * op + 2, :], -1.0e9, ps[:],
                        op0=MUL, op1=ADD)
                    nc.scalar.activation(pt_all[:, 2 * op:2 * op + 2, :],
                                         ps[:], AF.Exp)

                # QT first (its inputs are the first big DMAs), then each
                # KT fc chunk immediately followed by the two attention ot
                # pairs it unlocks — the PE is never more than one DMA
                # chunk ahead of the data stream, and the mask/exp tail
                # spreads across the whole projection phase instead of
                # bunching at its end.
                emit_qt()
                for fc in range(NDT):
                    for pt in range(NDT):
                        ps = psqk.tile([P, D], F32, name="psk", tag="psk")
                        for ct in range(NDT):
                            nc.tensor.matmul(
                                ps[:],
                                wkP[:, ct * D + pt * P:ct * D + (pt + 1) * P],
                                otP[:, fc * O + ct * D:fc * O + (ct + 1) * D],
                                start=(ct == 0), stop=(ct == NDT - 1))
                        nc.scalar.activation(
                            kt_sb[pt][:, fc * D:(fc + 1) * D].bitcast(F32R),
                            ps[:], AF.Identity,
                            bias=biasP[:, NDT + pt:NDT + pt + 1])
                    emit_attn(2 * fc)
                    emit_attn(2 * fc + 1)

            # ---- attnT, exp, rowsum -----------------------------------
            # ps4 (attn: 2 + rowsum: 2 banks) and pso (out: 4 banks) coexist
            # so the first batch's matmuls need not wait for the softmax
            # tail to release PSUM — otherwise the PE goes idle long enough
            # mid-kernel for the HAM clock-gate to re-throttle it.
            with tc.tile_pool(name="ps4", bufs=2, space="PSUM") as ps4:
                # ---- rowsum + mean term, interleaved per-ot so the
                # rowsum's LDWEIGHTS hide under the mean matmuls' streaming
                psr = [ps4.tile([P, 1], F32, name=f"psr{mt}", tag=f"psr{mt}",
                                bufs=1) for mt in range(NMT)]
                psm = [ps4.tile([P, D], F32, name=f"psm{mt}", tag=f"psm{mt}",
                                bufs=1) for mt in range(NMT)]
                for ot in range(NOT):
                    for mt in range(NMT):
                        nc.tensor.matmul(
                            psr[mt][:],
                            pt_all[:, ot, mt * P:(mt + 1) * P],
                            ones_sb[:],
                            start=(ot == 0), stop=(ot == NOT - 1))
                        nc.tensor.matmul(
                            psm[mt][:],
                            pt_all[:, ot, mt * P:(mt + 1) * P],
                            otherMP[:, ot, :],
                            start=(ot == 0), stop=(ot == NOT - 1))

                # ---- wf production + softmax epilogue -----------------
                # wf ops for batch b enter the (in-order) DVE/ACT queues
                # BEFORE the psum->SBUF copies of batch b-1, so a copy
                # stalled on the PE never blocks wf production.
                osb = {}
                wfs = {}

                NA = NOT - N_WF_DVE     # ACT chunks: 0..NA-1

                def emit_wf(b):
                    # two separate tiles so the DVE and ACT write streams
                    # have no common tile and never serialize on it.  ACT
                    # takes the FIRST chunks (their exps finish earliest,
                    # so ACT's strict-FIFO queue never blocks on a late
                    # exp); DVE's merged op takes the rest.
                    wfb = wpool.tile([P, NA, MC], F8, name="wfb", tag="wfb")
                    wfa = wpool.tile([P, N_WF_DVE, MC], F8, name="wfa",
                                     tag="wfa")
                    wfs[b] = (wfa, wfb)
                    for ot in range(NA):
                        nc.scalar.activation(
                            wfb[:, ot, :], pt_all[:, ot, :],
                            AF.Copy, scale=deltaP[:, ot, b:b + 1])
                    # DVE: one merged 3D op with stride-0 broadcast delta
                    nc.vector.tensor_tensor(
                        wfa[:], pt_all[:, NA:NOT, :],
                        deltaP[:, NA:NOT, b:b + 1]
                        .to_broadcast([P, N_WF_DVE, MC]), MUL)

                # wf for b=0,1 BEFORE the recip/mean ops: DVE/ACT chew them
                # during the rowsum/mean matmul window, and the recip/mean
                # ops (which agg b0 gates on) become ready right as psr/psm
                # complete.
                emit_wf(0)
                emit_wf(1)

                for mt in range(NMT):
                    nc.vector.reciprocal(recip_sb[mt][:], psr[mt][:])
                    nc.scalar.activation(mean_sb[mt][:], psm[mt][:], AF.Copy)

            # ---- weighted aggregation (fp8 DoubleRow) -----------------
            with tc.tile_pool(name="pso", bufs=8, space="PSUM") as psop:

                def emit_agg(b):
                    wfa, wfb = wfs.pop(b)
                    for mt in range(NMT):
                        if b % GB == 0:
                            osb[mt] = outp.tile([P, GB * D], BF16,
                                                name="osb", tag=f"osb{mt}")
                        ps = psop.tile([P, D], F32, name="pso", tag="pso")
                        # open the group with psum = mean (identity MM),
                        # then accumulate the fp8 delta-term on top
                        nc.tensor.matmul(ps[:], identP[:], mean_sb[mt][:],
                                         start=True, stop=False,
                                         skip_group_check=True)
                        for op in range(NOT // 2):
                            hi = 2 * op >= NA
                            w = wfa if hi else wfb
                            o0 = 2 * op - (NA if hi else 0)
                            nc.tensor.matmul(
                                ps[:],
                                w[:, o0:o0 + 2, mt * P:(mt + 1) * P],
                                otherP[:, 2 * op:2 * op + 2, :],
                                start=False, stop=(op == NOT // 2 - 1),
                                perf_mode=DR, skip_group_check=True)
                        j = b % GB
                        # BOTH copies on ACT: DVE's merged wf op is its
                        # whole per-batch budget (3.35us); ACT has slack.
                        nc.scalar.activation(
                            osb[mt][:, j * D:(j + 1) * D], ps[:],
                            AF.Copy, scale=recip_sb[mt][:])
                        # stores: mt0 via sync, mt1 via gpsimd (parallel
                        # issue queues).  Groups of GB until the tail; the
                        # last group goes pair+single+single so the final
                        # post-compute DMA is small and issues overlap.
                        eng = nc.sync if mt == 0 else nc.gpsimd
                        if b < B - GB:
                            if j == GB - 1:
                                eng.dma_start(
                                    d_out[mt * P:(mt + 1) * P,
                                          b - GB + 1:b + 1, :], osb[mt][:])
                        elif b == B - 3:
                            eng.dma_start(
                                d_out[mt * P:(mt + 1) * P, B - GB:B - 2, :],
                                osb[mt][:, 0:2 * D])
                        elif b >= B - 2:
                            eng.dma_start(
                                d_out[mt * P:(mt + 1) * P, b:b + 1, :],
                                osb[mt][:, j * D:(j + 1) * D])

                # emit_agg(b) BEFORE emit_wf(b+2): tile's PSUM-bank release
                # waits are engine-sem watermarks quantized to FIFO order,
                # so any wf TT emitted before an agg matmul lands in that
                # matmul's wait watermark — emitting lookahead wf ops first
                # made agg(0) transitively wait ~2 extra 3.4us DVE ops
                # (measured 3.9us PE gap at the agg transition).
                for b in range(B):
                    emit_agg(b)
                    if b + 2 < B:
                        emit_wf(b + 2)

    nc.compile()
    return nc


def _pack(a, ntiles, width):
    """[ntiles*128, width] -> [128, ntiles*width] partition-packed layout."""
    return np.ascontiguousarray(
        a.reshape(ntiles, P, width).transpose(1, 0, 2).reshape(P, -1))


def kernel(main_feat, other_feat, fix_feat, mask, Wq, bq, Wk, bk):
    global LAST_RESULTS
    main_feat = np.asarray(main_feat, dtype=np.float32)
    other_feat = np.asarray(other_feat, dtype=np.float32)
    fix_feat = np.asarray(fix_feat, dtype=np.float32)
    mask = np.asarray(mask)
    Wq = np.asarray(Wq, dtype=np.float32)
    bq = np.asarray(bq, dtype=np.float32)
    Wk = np.asarray(Wk, dtype=np.float32)
    bk = np.asarray(bk, dtype=np.float32)

    if "nc" not in _CACHE:
        _CACHE["nc"] = _build()
    nc = _CACHE["nc"]

    NDT, NOT = D // P, O // P
    inv = np.float32(1.0 / math.sqrt(D))
    wqT = _pack(Wq.T * inv, NDT, D).astype(ml_dtypes.bfloat16)
    bq_p = _pack((bq * inv).reshape(D, 1), NDT, 1)
    wkT = _pack(np.ascontiguousarray(Wk.T), NDT, D).astype(
        ml_dtypes.bfloat16)
    bk_p = _pack(bk.reshape(D, 1), NDT, 1)
    bias = np.concatenate([bq_p, bk_p], axis=1)       # [P, 2*NDT] f32
    # otherT fc-major: [p, fc*O + ct*D + oo] = other.T[ct*128+p, fc*D+oo]
    otherT = np.ascontiguousarray(
        other_feat.T.reshape(NDT, P, NDT, D).transpose(1, 2, 0, 3)
        .reshape(P, NDT * O)).astype(ml_dtypes.bfloat16)
    other8 = _pack(other_feat, NOT, D).astype(ml_dtypes.float8_e4m3)
    mu = fix_feat.mean(axis=0)                        # [O]
    otherM = _pack(mu[:, None] * other_feat, NOT, D).astype(
        ml_dtypes.bfloat16)
    deltaT = _pack(np.ascontiguousarray((fix_feat - mu).T), NOT, B)
    ident = np.eye(P, dtype=np.float32).astype(ml_dtypes.bfloat16)
    mainT = main_feat.T                               # [D, M] view
    mask_u8 = mask.astype(np.uint8)                   # [M, O]

    in_maps = []
    for c in range(N_CORES):
        sl = slice(c * MC, (c + 1) * MC)
        in_maps.append({
            "mainT": _pack(np.ascontiguousarray(mainT[:, sl]),
                           NDT, MC).astype(ml_dtypes.bfloat16),
            "wqT": wqT, "wkT": wkT, "bias": bias,
            "otherT": otherT, "other8": other8, "otherM": otherM,
            "deltaT": deltaT, "ident": ident,
            "maskT": _pack(np.ascontiguousarray(mask_u8[sl, :].T), NOT, MC),
        })

    try:
        res = run_bass_kernel_spmd(nc, in_maps, core_ids=list(range(N_CORES)))
    except Exception:
        # The BASS_TRACE=1 profiling path needs antenv.axon_hooks + artifact
        # upload, which not every image carries — rerun without tracing.
        if os.environ.get("BASS_NEVER_TRACE") == "1":
            raise
        os.environ["BASS_NEVER_TRACE"] = "1"
        res = run_bass_kernel_spmd(nc, in_maps, core_ids=list(range(N_CORES)))
    LAST_RESULTS = res
    # device layout is [MC, B, D] per core -> [B, MC, D], concat on m
    return np.concatenate(
        [res.results[c]["out"].transpose(1, 0, 2) for c in range(N_CORES)],
        axis=1).astype(np.float32)


# revision 13
# speedup vs baseline: 1.1795x; 1.0026x over previous
"""Trainium2 8-core kernel for nn_AttnAgg (sparse attention aggregation).

Math (see reference):
  Q = main @ Wq.T + bq                     [2048, 512]
  K = other @ Wk.T + bk                    [2048, 512]
  attn = softmax(where(mask, -BIG, Q K.T / sqrt(512)), axis=-1)   [2048, 2048]
  out[b, m, k] = sum_o attn[m, o] * fix[b, o] * other[o, k]       [32, 2048, 512]

Sharding: rows of `main` (the m axis) are split 256-per-core across 8 cores —
attention and the big einsum shard perfectly with zero collectives; only the
K projection (~1 GFLOP) is replicated.

The dominant cost is the batched aggregation einsum (B*M*O*D = 137 GFLOP of
the ~144 GFLOP total).  It runs in fp8e4 with perf_mode=DoubleRow (2 fp8
MACs per PE cell per cycle; the DR matmuls issue at the 512-cycle streaming
floor).  Straight fp8 fails the 2e-2 tolerance (measured 2.9e-2), so the
batch coupling `fix` is mean/delta decomposed on the host:
fix[b,o] = mu[o] + delta[b,o].  The batch-independent mu-term
(p @ bf16(mu*other)) is ONE extra bf16 matmul pass (1/32 of the einsum
work); only the delta-term runs in fp8, and |p*delta| is ~half |p*fix|,
which halves the fp8 noise (measured 1.29e-2).  The softmax denominator
comes from the same bf16 p (matmul with ones), so normalization is
consistent.  Projections run on bf16 inputs; the logits matmul stays
float32r (fp8 anywhere in the projections/logits measured >2e-2 in
numpy simulation — don't retry).

Per-batch steady state is balanced three ways at ~3.8us (measured op costs:
DVE merged TT = (elems+151)/0.96ns, ACT fp8-out chunk ~635ns, copies ~0.7us):
  PE:  1 identity mean-add MM (mt0 only) + 16 DoubleRow MMs  ~3.7us
  DVE: merged 3D tensor_tensor wf[0:11] = pt * delta (stride-0 broadcast
       delta column, runs at 1x - fp32/broadcast blocks the 2x uop), plus
       the mt1 output STT copy (ps*recip + meanR in ONE op)       ~3.8us
  ACT: wf[11:16] (5 per-chunk activations; fp8-out costs 2cyc/elem on ACT),
       plus the mt0 output copy                                    ~3.8us
mt1's mean term is added during the copy-out (scalar_tensor_tensor with
per-partition recip scalar and a precomputed f32 meanR = psm*recip), which
drops its identity matmul from the PE.  mt0 keeps the identity-MM trick
(ACT's activation cannot add a full-tensor bias).  GPSIMD measured 15x
slower than DVE for elementwise fp8 — not usable for wf.

Emission is software-pipelined LOOKAHEAD batches ahead; additionally wf for
batches 0-1 is emitted BEFORE the recip/mean copies so DVE/ACT produce them
during the ~7us rowsum/mean matmul window (engine queues are strict FIFO —
this ordering is load-bearing).

DMA: descriptor issue costs ~650ns per dma_start ON THE ISSUING ENGINE'S
QUEUE (measured), so input loads are consolidated into few, large DMAs and
spread across the sync/gpsimd/vector/scalar queues to issue in parallel
right after the ~6us engine-preamble barrier.  PE warmup matmuls (HAM
clock-gate) gate on a DVE memset tile, not on input DMA, so the ramp starts
at ~5.6us.  Output stores: GB-batch groups, mt0 issued from sync / mt1 from
gpsimd (parallel); the last group is split pair+single+single so the final
transfer is small and the serial ~650ns issues overlap earlier compute.

Inputs are fed pre-transposed AND partition-packed: every DRAM tensor is
laid out [128, *] so that each SBUF partition's data is one long contiguous
DRAM run.  A tile row-block T of a logical [T*128, W] matrix lives at
packed[:, T*W:(T+1)*W]; for DoubleRow the pair dim indexes adjacent 128-row
blocks of the contraction (o) axis.
"""

import math
import os
import sys

import numpy as np

if "/opt/trn_rl_repo" not in sys.path:
    sys.path.insert(0, "/opt/trn_rl_repo")

import ml_dtypes

import concourse.bass as bass
import concourse.tile as tile
from concourse import bacc, mybir
from concourse.bass_utils import run_bass_kernel_spmd

F32 = mybir.dt.float32
F32R = mybir.dt.float32r
BF16 = mybir.dt.bfloat16
F8 = mybir.dt.float8e4
U8 = mybir.dt.uint8
AF = mybir.ActivationFunctionType
DR = mybir.MatmulPerfMode.DoubleRow
MUL = mybir.AluOpType.mult
ADD = mybir.AluOpType.add

N_CORES = 8
M, O, D = 2048, 2048, 512       # main rows, other rows, qdim=kdim=mid
B = 32                          # batch
MC = M // N_CORES               # 256 main rows per core
P = 128
GB = 4                          # batches per output store DMA
N_WARM = 30                     # dummy matmuls to warm the PE clock gate
N_WF_DVE = 12                   # wf chunks (of 16) on DVE (one merged op)
LOOKAHEAD = 5                   # extra wf batches beyond the 2 pre-produced
TAIL_PB = 8                     # per-batch output stores for the last 8

_CACHE = {}
LAST_RESULTS = None             # test harness reads exec_time_ns from here


def _build():
    nc = bacc.Bacc("TRN2", target_bir_lowering=False, debug=False,
                   num_devices=N_CORES)

    NDT = D // P                # 4 tiles along the 512 dims
    NOT = O // P                # 16 tiles along o
    NMT = MC // P               # 2 tiles along m

    d_mainT = nc.dram_tensor("mainT", [P, NDT * MC], BF16,
                             kind="ExternalInput").ap()
    d_wqT = nc.dram_tensor("wqT", [P, NDT * D], BF16,
                           kind="ExternalInput").ap()
    d_wkT = nc.dram_tensor("wkT", [P, NDT * D], BF16,
                           kind="ExternalInput").ap()
    d_bias = nc.dram_tensor("bias", [P, 2 * NDT], F32,
                            kind="ExternalInput").ap()   # bq || bk
    d_otherT = nc.dram_tensor("otherT", [P, NDT * O], BF16,
                              kind="ExternalInput").ap()   # fc-major
    d_other8 = nc.dram_tensor("other8", [P, NOT * D], F8,
                              kind="ExternalInput").ap()   # ot-major, fp8
    d_otherM = nc.dram_tensor("otherM", [P, NOT * D], BF16,
                              kind="ExternalInput").ap()   # mu*other, bf16
    d_deltaT = nc.dram_tensor("deltaT", [P, NOT * B], F32,
                              kind="ExternalInput").ap()   # fix - mu
    d_maskT = nc.dram_tensor("maskT", [P, NOT * MC], U8,
                             kind="ExternalInput").ap()
    d_ident = nc.dram_tensor("ident", [P, P], BF16,
                             kind="ExternalInput").ap()
    d_out = nc.dram_tensor("out", [MC, B, D], BF16,
                           kind="ExternalOutput").ap()

    with tile.TileContext(nc) as tc:
        with tc.tile_pool(name="persist", bufs=1) as pp, \
             tc.tile_pool(name="wpool", bufs=16) as wpool, \
             tc.tile_pool(name="outp", bufs=4) as outp:

            # ---- loads: few big DMAs, issued in parallel across four
            # engine queues (each dma_start costs ~650ns of issue time on
            # its queue).  Ordered by need-time within each queue. --------
            with tc.tile_pool(name="proj", bufs=1) as proj, \
                 tc.tile_pool(name="psqk", bufs=2, space="PSUM") as psqk:
                # ---- PE warmup (emitted FIRST so the DVE memset is the
                # head of the vector queue): dummy matmuls gated on a
                # memset tile, NOT on input DMA — the HAM ramp starts
                # right after the engine preamble (~5.6us), so the
                # clock-gate is at 8/8 before real work begins.
                warmP = proj.tile([P, P], BF16, name="warmP", tag="warmP")
                nc.vector.memset(warmP[:], 0.125)
                # warm_ps shares the psq tag's buffer ring — a dedicated
                # tag would need a 9th PSUM bank (psq2+psk2+psa2+psr2=8)
                warm_ps = psqk.tile([P, P], F32, name="warm_ps", tag="psq")
                for _ in range(N_WARM):
                    nc.tensor.matmul(warm_ps[:], warmP[:], warmP[:],
                                     start=True, stop=True)

                # ALL input loads on the single sync queue, strictly in
                # need order: concurrent multi-queue input DMAs measured
                # SLOWER — they flood HBM and the critical-path tensors
                # (wk/ot for the K projection) drop to a 1/3 bandwidth
                # share, starving the PE and re-throttling the HAM clock
                # gate.  Serial issue on one queue = transfers get full
                # bandwidth in exactly this order.  fc/ct-granular splits
                # give the K projection chunk-level semaphores so it can
                # start/progress as data lands.
                biasP = proj.tile([P, 2 * NDT], F32, name="biasP",
                                  tag="biasP")
                nc.sync.dma_start(biasP[:], d_bias[:])
                wqP = proj.tile([P, NDT * D], BF16, name="wqP", tag="wqP")
                nc.sync.dma_start(wqP[:], d_wqT[:])
                mtP = proj.tile([P, NDT * MC], BF16, name="mtP", tag="mtP")
                nc.sync.dma_start(mtP[:], d_mainT[:])
                wkP = proj.tile([P, NDT * D], BF16, name="wkP", tag="wkP")
                nc.sync.dma_start(wkP[:], d_wkT[:])
                otP = proj.tile([P, NDT * O], BF16, name="otP", tag="otP")
                for ct in range(NDT):
                    nc.sync.dma_start(otP[:, ct * D:(ct + 1) * D],
                                      d_otherT[:, ct * D:(ct + 1) * D])
                for fc in range(1, NDT):
                    nc.sync.dma_start(otP[:, fc * O:(fc + 1) * O],
                                      d_otherT[:, fc * O:(fc + 1) * O])
                maskP = pp.tile([P, NOT, MC], U8, name="maskP", tag="maskP")
                nc.sync.dma_start(maskP[:], d_maskT[:])
                otherMP = pp.tile([P, NOT, D], BF16, name="otherMP",
                                  tag="otherMP")
                nc.sync.dma_start(otherMP[:], d_otherM[:])
                otherP = pp.tile([P, NOT, D], F8, name="otherP",
                                 tag="otherP")
                nc.sync.dma_start(otherP[:], d_other8[:])
                deltaP = pp.tile([P, NOT, B], F32, name="deltaP",
                                 tag="deltaP")
                nc.sync.dma_start(deltaP[:], d_deltaT[:])
                identP = pp.tile([P, P], BF16, name="identP", tag="identP")
                nc.sync.dma_start(identP[:], d_ident[:])

                psr = [psqk.tile([P, 1], F32, name=f"psr{mt}",
                                 tag=f"psr{mt}", bufs=1) for mt in range(NMT)]
                qt_sb = [pp.tile([P, MC], F32, name=f"qt{i}", tag=f"qt{i}")
                         for i in range(NDT)]
                kt_sb = [pp.tile([P, O], F32, name=f"kt{i}", tag=f"kt{i}")
                         for i in range(NDT)]
                pt_all = pp.tile([P, NOT, MC], BF16, name="pt", tag="pt")
                ones_sb = pp.tile([P, 1], BF16, name="ones", tag="ones")
                nc.vector.memset(ones_sb[:], 1.0)
                recip_sb = [pp.tile([P, 1], F32, name=f"recip{i}",
                                    tag=f"recip{i}") for i in range(NMT)]
                mean_sb = [pp.tile([P, D], BF16, name=f"mean{i}",
                                   tag=f"mean{i}") for i in range(NMT)]

                # ---- KT fc0 first (earliest DMAs), then QT, then rest
                def emit_qt():
                    for pt in range(NDT):
                        ps = psqk.tile([P, MC], F32, name="psq", tag="psq")
                        for ct in range(NDT):
                            nc.tensor.matmul(
                                ps[:],
                                wqP[:, ct * D + pt * P:ct * D + (pt + 1) * P],
                                mtP[:, ct * MC:(ct + 1) * MC],
                                start=(ct == 0), stop=(ct == NDT - 1))
                        nc.scalar.activation(qt_sb[pt][:].bitcast(F32R),
                                             ps[:], AF.Identity,
                                             bias=biasP[:, pt:pt + 1])

                def emit_attn(op):
                    # logits for ot pair (2op, 2op+1), mask, exp
                    ps = psqk.tile([P, 2, MC], F32, name="psa", tag="psa")
                    for j in range(2):
                        ot = 2 * op + j
                        for ct in range(NDT):
                            nc.tensor.matmul(
                                ps[:, j, :],
                                kt_sb[ct][:, ot * P:(ot + 1) * P]
                                .bitcast(F32R),
                                qt_sb[ct][:].bitcast(F32R),
                                start=(ct == 0), stop=(ct == NDT - 1))
                    # psa += mask * -1e9  (u8 -> f32 convert, scale, add in
                    # one DVE pass); exp underflows masked lanes to exact 0
                    nc.vector.scalar_tensor_tensor(
                        ps[:], maskP[:, 2 * op:2 * op + 2, :], -1.0e9, ps[:],
                        op0=MUL, op1=ADD)
                    nc.scalar.activation(pt_all[:, 2 * op:2 * op + 2, :],
                                         ps[:], AF.Exp)

                def emit_rowsum(op):
                    # rowsum of pt pair (2op, 2op+1): tiny ones-matmuls,
                    # LDWEIGHTS hides under neighboring attention streams.
                    for j in range(2):
                        ot = 2 * op + j
                        for mt in range(NMT):
                            nc.tensor.matmul(
                                psr[mt][:],
                                pt_all[:, ot, mt * P:(mt + 1) * P],
                                ones_sb[:],
                                start=(ot == 0), stop=(ot == NOT - 1))

                # QT first (its inputs are the first big DMAs), then each
                # KT fc chunk immediately followed by the two attention ot
                # pairs it unlocks — the PE is never more than one DMA
                # chunk ahead of the data stream, and the mask/exp tail
                # spreads across the whole projection phase instead of
                # bunching at its end.
                emit_qt()
                for fc in range(NDT):
                    for pt in range(NDT):
                        ps = psqk.tile([P, D], F32, name="psk", tag="psk")
                        for ct in range(NDT):
                            nc.tensor.matmul(
                                ps[:],
                                wkP[:, ct * D + pt * P:ct * D + (pt + 1) * P],
                                otP[:, fc * O + ct * D:fc * O + (ct + 1) * D],
                                start=(ct == 0), stop=(ct == NDT - 1))
                        nc.scalar.activation(
                            kt_sb[pt][:, fc * D:(fc + 1) * D].bitcast(F32R),
                            ps[:], AF.Identity,
                            bias=biasP[:, NDT + pt:NDT + pt + 1])
                    for op in (2 * fc, 2 * fc + 1):
                        # rowsum lags the attention by TWO ot-pairs so its
                        # exp input is always ready when the PE reaches it
                        # (lag-1 would stall ~1.4us/op on the mask+exp
                        # latency); the tail pairs flush after the loop.
                        if op >= 2:
                            emit_rowsum(op - 2)
                        emit_attn(op)
                emit_rowsum(NOT // 2 - 2)
                emit_rowsum(NOT // 2 - 1)

            # ---- attnT, exp, rowsum -----------------------------------
            # ps4 (attn: 2 + rowsum: 2 banks) and pso (out: 4 banks) coexist
            # so the first batch's matmuls need not wait for the softmax
            # tail to release PSUM — otherwise the PE goes idle long enough
            # mid-kernel for the HAM clock-gate to re-throttle it.
            with tc.tile_pool(name="ps4", bufs=2, space="PSUM") as ps4:
                # recips FIRST: psr completed with the attention phase, so
                # the reciprocals are ready immediately and can never queue
                # behind a 3.4us wf TT on the DVE (that ordering cost a
                # 3.2us PE stall at the agg transition when rowsum/recip
                # ran after the mean matmuls).
                for mt in range(NMT):
                    nc.vector.reciprocal(recip_sb[mt][:], psr[mt][:])
                psm = [ps4.tile([P, D], F32, name=f"psm{mt}", tag=f"psm{mt}",
                                bufs=1) for mt in range(NMT)]
                for ot in range(NOT):
                    for mt in range(NMT):
                        nc.tensor.matmul(
                            psm[mt][:],
                            pt_all[:, ot, mt * P:(mt + 1) * P],
                            otherMP[:, ot, :],
                            start=(ot == 0), stop=(ot == NOT - 1))

                # ---- wf production + softmax epilogue -----------------
                # wf ops for batch b enter the (in-order) DVE/ACT queues
                # BEFORE the psum->SBUF copies of batch b-1, so a copy
                # stalled on the PE never blocks wf production.
                osb = {}
                wfs = {}

                NA = NOT - N_WF_DVE     # ACT chunks: 0..NA-1

                def emit_wf(b):
                    # two separate tiles so the DVE and ACT write streams
                    # have no common tile and never serialize on it.  ACT
                    # takes the FIRST chunks (their exps finish earliest,
                    # so ACT's strict-FIFO queue never blocks on a late
                    # exp); DVE's merged op takes the rest.
                    wfb = wpool.tile([P, NA, MC], F8, name="wfb", tag="wfb")
                    wfa = wpool.tile([P, N_WF_DVE, MC], F8, name="wfa",
                                     tag="wfa")
                    wfs[b] = (wfa, wfb)
                    for ot in range(NA):
                        nc.scalar.activation(
                            wfb[:, ot, :], pt_all[:, ot, :],
                            AF.Copy, scale=deltaP[:, ot, b:b + 1])
                    # DVE: one merged 3D op with stride-0 broadcast delta
                    nc.vector.tensor_tensor(
                        wfa[:], pt_all[:, NA:NOT, :],
                        deltaP[:, NA:NOT, b:b + 1]
                        .to_broadcast([P, N_WF_DVE, MC]), MUL)

                # wf for b=0,1 BEFORE the recip/mean ops: DVE/ACT chew them
                # during the rowsum/mean matmul window, and the recip/mean
                # ops (which agg b0 gates on) become ready right as psr/psm
                # complete.
                emit_wf(0)
                emit_wf(1)

                for mt in range(NMT):
                    nc.scalar.activation(mean_sb[mt][:], psm[mt][:], AF.Copy)

            # ---- weighted aggregation (fp8 DoubleRow) -----------------
            with tc.tile_pool(name="pso", bufs=8, space="PSUM") as psop:

                def emit_agg(b):
                    wfa, wfb = wfs.pop(b)
                    for mt in range(NMT):
                        if b % GB == 0:
                            osb[mt] = outp.tile([P, GB * D], BF16,
                                                name="osb", tag=f"osb{mt}")
                        ps = psop.tile([P, D], F32, name="pso", tag="pso")
                        # open the group with psum = mean (identity MM),
                        # then accumulate the fp8 delta-term on top
                        nc.tensor.matmul(ps[:], identP[:], mean_sb[mt][:],
                                         start=True, stop=False,
                                         skip_group_check=True)
                        for op in range(NOT // 2):
                            hi = 2 * op >= NA
                            w = wfa if hi else wfb
                            o0 = 2 * op - (NA if hi else 0)
                            nc.tensor.matmul(
                                ps[:],
                                w[:, o0:o0 + 2, mt * P:(mt + 1) * P],
                                otherP[:, 2 * op:2 * op + 2, :],
                                start=False, stop=(op == NOT // 2 - 1),
                                perf_mode=DR, skip_group_check=True)
                        j = b % GB
                        # BOTH copies on ACT: DVE's merged wf op is its
                        # whole per-batch budget (3.35us); ACT has slack.
                        nc.scalar.activation(
                            osb[mt][:, j * D:(j + 1) * D], ps[:],
                            AF.Copy, scale=recip_sb[mt][:])
                        # stores: mt0 via sync, mt1 via gpsimd (parallel
                        # issue queues).  Groups of GB until the tail; the
                        # last group goes pair+single+single so the final
                        # post-compute DMA is small and issues overlap.
                        eng = nc.sync if mt == 0 else nc.gpsimd
                        if b < B - GB:
                            if j == GB - 1:
                                eng.dma_start(
                                    d_out[mt * P:(mt + 1) * P,
                                          b - GB + 1:b + 1, :], osb[mt][:])
                        elif b == B - 3:
                            eng.dma_start(
                                d_out[mt * P:(mt + 1) * P, B - GB:B - 2, :],
                                osb[mt][:, 0:2 * D])
                        elif b >= B - 2:
                            eng.dma_start(
                                d_out[mt * P:(mt + 1) * P, b:b + 1, :],
                                osb[mt][:, j * D:(j + 1) * D])

                # emit_agg(b) BEFORE emit_wf(b+2): tile's PSUM-bank release
                # waits are engine-sem watermarks quantized to FIFO order,
                # so any wf TT emitted before an agg matmul lands in that
                # matmul's wait watermark — emitting lookahead wf ops first
                # made agg(0) transitively wait ~2 extra 3.4us DVE ops
                # (measured 3.9us PE gap at the agg transition).
                for b in range(B):
                    emit_agg(b)
                    if b + 2 < B:
                        emit_wf(b + 2)

    nc.compile()
    return nc


def _pack(a, ntiles, width):
    """[ntiles*128, width] -> [128, ntiles*width] partition-packed layout."""
    return np.ascontiguousarray(
        a.reshape(ntiles, P, width).transpose(1, 0, 2).reshape(P, -1))


def kernel(main_feat, other_feat, fix_feat, mask, Wq, bq, Wk, bk):
    global LAST_RESULTS
    main_feat = np.asarray(main_feat, dtype=np.float32)
    other_feat = np.asarray(other_feat, dtype=np.float32)
    fix_feat = np.asarray(fix_feat, dtype=np.float32)
    mask = np.asarray(mask)
    Wq = np.asarray(Wq, dtype=np.float32)
    bq = np.asarray(bq, dtype=np.float32)
    Wk = np.asarray(Wk, dtype=np.float32)
    bk = np.asarray(bk, dtype=np.float32)

    if "nc" not in _CACHE:
        _CACHE["nc"] = _build()
    nc = _CACHE["nc"]

    NDT, NOT = D // P, O // P
    inv = np.float32(1.0 / math.sqrt(D))
    wqT = _pack(Wq.T * inv, NDT, D).astype(ml_dtypes.bfloat16)
    bq_p = _pack((bq * inv).reshape(D, 1), NDT, 1)
    wkT = _pack(np.ascontiguousarray(Wk.T), NDT, D).astype(
        ml_dtypes.bfloat16)
    bk_p = _pack(bk.reshape(D, 1), NDT, 1)
    bias = np.concatenate([bq_p, bk_p], axis=1)       # [P, 2*NDT] f32
    # otherT fc-major: [p, fc*O + ct*D + oo] = other.T[ct*128+p, fc*D+oo]
    otherT = np.ascontiguousarray(
        other_feat.T.reshape(NDT, P, NDT, D).transpose(1, 2, 0, 3)
        .reshape(P, NDT * O)).astype(ml_dtypes.bfloat16)
    other8 = _pack(other_feat, NOT, D).astype(ml_dtypes.float8_e4m3)
    mu = fix_feat.mean(axis=0)                        # [O]
    otherM = _pack(mu[:, None] * other_feat, NOT, D).astype(
        ml_dtypes.bfloat16)
    deltaT = _pack(np.ascontiguousarray((fix_feat - mu).T), NOT, B)
    ident = np.eye(P, dtype=np.float32).astype(ml_dtypes.bfloat16)
    mainT = main_feat.T                               # [D, M] view
    mask_u8 = mask.astype(np.uint8)                   # [M, O]

    in_maps = []
    for c in range(N_CORES):
        sl = slice(c * MC, (c + 1) * MC)
        in_maps.append({
            "mainT": _pack(np.ascontiguousarray(mainT[:, sl]),
                           NDT, MC).astype(ml_dtypes.bfloat16),
            "wqT": wqT, "wkT": wkT, "bias": bias,
            "otherT": otherT, "other8": other8, "otherM": otherM,
            "deltaT": deltaT, "ident": ident,
            "maskT": _pack(np.ascontiguousarray(mask_u8[sl, :].T), NOT, MC),
        })

    try:
        res = run_bass_kernel_spmd(nc, in_maps, core_ids=list(range(N_CORES)))
    except Exception:
        # The BASS_TRACE=1 profiling path needs antenv.axon_hooks + artifact
        # upload, which not every image carries — rerun without tracing.
        if os.environ.get("BASS_NEVER_TRACE") == "1":
            raise
        os.environ["BASS_NEVER_TRACE"] = "1"
        res = run_bass_kernel_spmd(nc, in_maps, core_ids=list(range(N_CORES)))
    LAST_RESULTS = res
    # device layout is [MC, B, D] per core -> [B, MC, D], concat on m
    return np.concatenate(
        [res.results[c]["out"].transpose(1, 0, 2) for c in range(N_CORES)],
        axis=1).astype(np.float32)
